# revision 6
# baseline (speedup 1.0000x reference)
"""Trainium2 Bass kernel for nn_BPBookMemory (retrieval_knn), v3.

Strategy (8 NeuronCores, SPMD, data-parallel batch + memory-shard):
  - x sharded by batch (8/core), memory sharded 8-way (8192 rows/core).
  - Warmup AllReduce triggered at t~0 over an uninitialized DRAM buffer:
    the first collective pays a fixed ~60-70us cold cost (ncfw barrier +
    first op); triggering it immediately hides all of it under phase A.
  - x batches AND memory chunks both ride the gpsimd SWDGE cast-DMA
    queue (fp32 HBM -> bf16 SBUF), interleaved, probe-gated so arrivals
    stay ordered and phase A streams.
  - Phase A: per batch, 4 groups of 8 PE transposes into one PSUM tile
    (fewer, denser PE ops than per-4 groups), DVE copy, conv matmuls,
    gelu+accum -> q sums. Memory norm/normalize/transpose interleaved.
  - q is normalized LOCALLY (per-core rsqrt + column scale) before the
    AllGather, so the gathered q columns are unit-norm: no post-AG cinv
    chain and no per-partition exp scales anywhere downstream.
  - sim = qcb^T @ m_hat_T (bf16, fold-2 into [128, 4096]); block max8
    pipelined per 1024 cols -> 64 candidates/batch/core; AllGather;
    merge via max8+match_replace. Dense exp during the AllGather flight.
  - W = expw * (sim >= thresh) via two bulk [128,4096] DVE ops; W^T via
    32 full fold-2 PE transposes (each yields lhsT for tiles k and
    k+32); proto accumulated in PSUM over 64 matmuls; scaled by
    retrieval_scale/Z_b; ReduceScatter; phase E: out = x + proto
    broadcast, bf16 adds, SWDGE cast-DMA stores.

Index-free top-k: only candidate VALUES travel; selection is by
threshold (sim >= 16th-largest), so no gather is needed.
"""

import os
import sys

for _p in ("/opt/trn_rl_repo", "/root/.axon_site/_ro/trn_rl_repo"):
    if os.path.isdir(_p) and _p not in sys.path:
        sys.path.append(_p)

import numpy as np
from contextlib import ExitStack

import concourse.bass as bass
import concourse.tile as tile
from concourse import mybir
from concourse.bass_utils import run_bass_kernel_spmd
from concourse.vector_clock import ScopedClock

F32 = mybir.dt.float32
BF16 = mybir.dt.bfloat16
AF = mybir.ActivationFunctionType
ALU = mybir.AluOpType

NCORES = 8
B, N, D, S = 64, 4096, 128, 65536
BL = B // NCORES          # 8 batches per core
SL = S // NCORES          # 8192 memory rows per core
TPB = N // 128            # 32 x tiles per batch
MT = SL // 128            # 64 memory 128-row tiles per core
MC = SL // 512            # 16 memory chunks of 512 rows


# ---------------------------------------------------------------------------
# Walrus workaround: this container's neuronxcc rejects instructions carrying
# more than ~1 sync wait command (Drain/TPB_CTRL, LDWEIGHTS/S3_LW...).
# 1) Replace Tile's exit drain+barrier with EventSemaphore-carried waits.
# 2) Post-pass: hoist excess waits onto standalone EventSemaphore insts.
# ---------------------------------------------------------------------------

def _patched_drain_and_barrier(self, tick_clock, wait_clock):
    nc = self.nc
    carrier = nc.sync.add_instruction(
        mybir.InstEventSemaphore(name=f"I-{nc.next_id()}", ins=[], outs=[])
    )
    wait_clock.add_sem_waits(carrier.ins, ScopedClock({None: tick_clock.global_clock}))
    si = carrier.ins.sync_info
    waits = list(si.on_wait or [])
    if len(waits) > 1:
        si.on_wait = [waits[0]]
        for w in waits[1:]:
            extra = nc.sync.add_instruction(
                mybir.InstEventSemaphore(name=f"I-{nc.next_id()}", ins=[], outs=[])
            )
            extra.ins.sync_info = mybir.SyncInfo(on_wait=[w], on_update=[])
    for eng in nc.engines.values():
        eng.drain()
    nc.all_engine_barrier(sem_only=True)
    popped = nc._tile_sem_poison_stack.pop()
    assert popped is self._sem_poison
    nc.clear_and_free_semaphores(list(self.sems.allocated().values()))
    nc.all_engine_barrier(sem_only=True)


tile.TileContext._drain_and_barrier = _patched_drain_and_barrier

_hoist_ctr = [0]


def _hoist_waits(nc, max_keep=1):
    for f in nc.m.functions:
        for bb in f.blocks:
            insts = bb.instructions
            out = []
            changed = False
            for inst in insts:
                si = inst.sync_info
                waits = list(si.on_wait) if (si is not None and si.on_wait) else []
                if waits:
                    keep = 0 if inst.opcode == "Drain" else max_keep
                    kept, hoisted = [], []
                    for w in waits:
                        if len(kept) < keep and w.wait_mode == "sem-ge-imm":
                            kept.append(w)
                        else:
                            hoisted.append(w)
                    if hoisted:
                        for w in hoisted:
                            _hoist_ctr[0] += 1
                            ev = mybir.InstEventSemaphore(
                                name=f"I-hoistw-{_hoist_ctr[0]}", ins=[], outs=[]
                            )
                            ev.engine = inst.engine
                            ev.sync_info = mybir.SyncInfo(on_wait=[w], on_update=[])
                            out.append(ev)
                        si.on_wait = kept
                        changed = True
                out.append(inst)
            if changed:
                bb.instructions = out


# ---------------------------------------------------------------------------
# Helpers
# ---------------------------------------------------------------------------

def _rsqrt(nc, r, y, t, init_a, init_b, iters=3, eng=None):
    """r = 1/sqrt(y) elementwise via linear init + Newton (float ops only).

    r, y, t: same-shape fp32 APs (t is scratch; y preserved; r != y).
    Init r0 = init_a + init_b*y must land within ~+-40% of 1/sqrt(y) over
    the expected y range for `iters` Newton steps to converge."""
    if eng is None:
        eng = nc.vector
    eng.tensor_scalar(r, y, init_b, init_a, op0=ALU.mult, op1=ALU.add)
    for _ in range(iters):
        eng.tensor_tensor(t, r, r, op=ALU.mult)
        eng.tensor_tensor(t, t, y, op=ALU.mult)
        eng.tensor_scalar(t, t, -0.5, 1.5, op0=ALU.mult, op1=ALU.add)
        eng.tensor_tensor(r, r, t, op=ALU.mult)


# ---------------------------------------------------------------------------
# Kernel build
# ---------------------------------------------------------------------------

def build_program(debug=False):
    nc = bass.Bass(num_devices=NCORES)
    groups = [list(range(NCORES))]

    import concourse.tile_utils as tile_utils
    if getattr(tile_utils, "max_sbuf_usage", 0) < 200 * 1024:
        tile_utils.max_sbuf_usage = 200 * 1024

    xs = nc.dram_tensor("xs", [BL, N, D], F32, kind="ExternalInput")
    ms = nc.dram_tensor("ms", [SL, D], F32, kind="ExternalInput")
    convw = nc.dram_tensor("convw", [D, D], F32, kind="ExternalInput")
    convb = nc.dram_tensor("convb", [D], F32, kind="ExternalInput")
    scal = nc.dram_tensor("scal", [1], F32, kind="ExternalInput")
    ident_in = nc.dram_tensor("ident", [128, 128], F32, kind="ExternalInput")
    identb_in = nc.dram_tensor("identb", [128, 128], BF16, kind="ExternalInput")
    out_ext = nc.dram_tensor("out", [BL, N, D], F32, kind="ExternalOutput")

    # collective bounce buffers
    warm_in = nc.dram_tensor("warm_in", [8, 4], F32)
    warm_out = nc.dram_tensor("warm_out", [8, 4], F32, addr_space="Shared")
    q_in = nc.dram_tensor("q_in", [128, BL], F32)
    q_ag = nc.dram_tensor("q_ag", [128 * NCORES, BL], F32, addr_space="Shared")
    cand_in = nc.dram_tensor("cand_in", [B, 64], F32)
    cand_ag = nc.dram_tensor("cand_ag", [B * NCORES, 64], F32, addr_space="Shared")
    proto_in = nc.dram_tensor("proto_in", [B, D], F32)
    proto_rs = nc.dram_tensor("proto_rs", [BL, D], F32)

    with tile.TileContext(nc) as tc, ExitStack() as top:
        cst = top.enter_context(tc.tile_pool(name="cst", bufs=1))
        big = top.enter_context(tc.tile_pool(name="big", bufs=1))
        sml = top.enter_context(tc.tile_pool(name="sml", bufs=1))

        # ---- warmup collective FIRST: content is irrelevant, only the
        # ncfw cold-start (barrier + first-op init, ~60-70us) matters.
        nc.gpsimd.collective_compute(
            "AllReduce", ALU.add, replica_groups=groups,
            ins=[warm_in[:]], outs=[warm_out[:]],
        )

        # constants on the sync (HWDGE) queue
        ident = cst.tile([128, 128], F32)
        nc.sync.dma_start(ident[:], ident_in[:])
        identb = cst.tile([128, 128], BF16)
        nc.sync.dma_start(identb[:], identb_in[:])
        wconv = cst.tile([128, 128], F32)
        nc.sync.dma_start(wconv[:], convw[:])
        bias_col = cst.tile([128, 1], F32)
        nc.sync.dma_start(bias_col[:], convb[:].rearrange("(p o) -> p o", o=1))
        scal_sb = cst.tile([1, 1], F32)
        nc.sync.dma_start(scal_sb[:], scal[:].rearrange("(p o) -> p o", o=1))

        ones = cst.tile([128, 128], F32)
        nc.gpsimd.memset(ones[:], 1.0)
        wt_conv = cst.tile([128, 128], BF16)
        scal_col = cst.tile([128, 1], F32)

        # persistent SBUF
        xb = [big.tile([128, TPB, 128], BF16, name=f"xb{b}", tag=f"xb{b}")
              for b in range(BL)]                      # 8 KB/part each
        mraw = big.tile([128, MT, 128], BF16)          # 16 KB/part
        mhT = big.tile([128, MT, 128], BF16)           # 16 KB/part
        sim_sb = big.tile([128, 4096], BF16)           # 8 KB/part (fold-2)
        expw = big.tile([128, 4096], BF16)             # 8 KB/part
        qacc = sml.tile([128, 32], F32)
        qT_all = sml.tile([128, B], F32)
        qTb = sml.tile([128, B], BF16)
        nrm2 = sml.tile([128, MT], F32)
        minv = sml.tile([128, MT], F32)
        mscr = sml.tile([128, MT], F32)
        sqs = sml.tile([128, 32, 128], BF16)
        cands = sml.tile([128, 32], F32)
        cand_all = sml.tile([128, NCORES * 64], F32)
        mr_scr = sml.tile([128, NCORES * 64], F32)
        t16 = sml.tile([128, 16], F32)
        e16 = sml.tile([64, 16], F32)
        proto_sb = sml.tile([B, D], F32)
        proto_loc = sml.tile([1, BL * D], F32)
        cw = sml.tile([64, 8], F32)
        cwr = sml.tile([1, 64], F32)
        dume = sml.tile([1, 1], F32)
        xprobe = sml.tile([128, BL + 1], F32)

        # W^T for the conv (needs wconv arrival; PE ~idle at this point)
        with tc.tile_pool(name="wt0ps", bufs=1, space="PSUM") as wt0ps:
            wtp = wt0ps.tile([128, 128], F32)
            nc.tensor.transpose(wtp[:], wconv[:], ident[:])
            nc.vector.tensor_copy(wt_conv[:], wtp[:])
            scp = wt0ps.tile([128, 1], F32)
            nc.tensor.matmul(scp[:], ones[0:1, :], scal_sb[0:1, 0:1],
                             start=True, stop=True)
            nc.vector.tensor_copy(scal_col[:], scp[:])

        # ---- load stream: x batches + memory chunks interleaved on the
        # gpsimd SWDGE cast-DMA queue (fp32 HBM -> bf16 SBUF). Probes
        # keep <=2 x DMAs in flight so arrivals stay ordered.
        def issue_x(b):
            nc.gpsimd.dma_start(
                xb[b][:], xs[b].rearrange("(p t) d -> p t d", p=128)
            )

        def issue_ms(c):
            nc.gpsimd.dma_start(
                mraw[:, c * 4:(c + 1) * 4],
                ms[c * 512:(c + 1) * 512].rearrange("(p t) d -> p t d",
                                                    p=128),
            )

        issue_x(0)
        issue_x(1)
        issue_ms(0)
        issue_ms(1)
        for b in range(2, BL):
            # probe: gate issue of batch b on batch b-2's full arrival
            nc.gpsimd.tensor_copy(xprobe[:, b - 2:b - 1],
                                  xb[b - 2][:, 0, 0:1])
            issue_x(b)
            issue_ms(2 * b - 2)
            issue_ms(2 * b - 1)
        issue_ms(14)
        issue_ms(15)

        # ---- Phase A + memory prep, interleaved --------------------------
        with ExitStack() as pa:
            xp_ps = pa.enter_context(tc.tile_pool(name="xp_ps", bufs=2,
                                                  space="PSUM"))
            mp_ps = pa.enter_context(tc.tile_pool(name="mp_ps", bufs=1,
                                                  space="PSUM"))
            ft_ps = pa.enter_context(tc.tile_pool(name="ft_ps", bufs=2,
                                                  space="PSUM"))
            xt_sbp = pa.enter_context(tc.tile_pool(name="xt_sb", bufs=3))
            mn_p = pa.enter_context(tc.tile_pool(name="mn", bufs=2))
            gl_p = pa.enter_context(tc.tile_pool(name="gl", bufs=2))

            def mchunk_sq(c):
                # squares for row norms (DVE), from the bf16 rows
                nc.vector.tensor_tensor(
                    sqs[:, (c % 8) * 4:(c % 8) * 4 + 4],
                    mraw[:, c * 4:(c + 1) * 4],
                    mraw[:, c * 4:(c + 1) * 4], op=ALU.mult,
                )

            def mchunk_normalize(c):
                # normalized tiles (DVE) + PE transpose into mhT (ACT copy)
                mh = mn_p.tile([128, 4, 128], BF16, name=f"mh{c}", tag="mh")
                for t in range(4):
                    k = c * 4 + t
                    nc.vector.tensor_scalar_mul(
                        mh[:, t], mraw[:, k], minv[:, k:k + 1]
                    )
                mp = mp_ps.tile([128, 512], BF16, name=f"mp{c}", tag="mp")
                for t in range(4):
                    nc.tensor.transpose(
                        mp[:, t * 128:(t + 1) * 128], mh[:, t], identb[:]
                    )
                nc.scalar.copy(
                    mhT[:, c * 4:(c + 1) * 4],
                    mp[:].rearrange("p (t d) -> p t d", d=128),
                )

            for b in range(BL):
                # x tiles for batch b: 4 groups of 8 tiles; dense PE
                # transpose bursts (identity stays stationary per burst)
                for g in range(4):
                    xp = xp_ps.tile([128, 1024], BF16, name=f"xp{b}_{g}",
                                    tag="xp")
                    for i in range(8):
                        nc.tensor.transpose(
                            xp[:, i * 128:(i + 1) * 128],
                            xb[b][:, g * 8 + i], identb[:],
                        )
                    xsb = xt_sbp.tile([128, 1024], BF16)
                    nc.vector.tensor_copy(xsb[:], xp[:])
                    fps = ft_ps.tile([128, 1024], F32, name=f"fp{b}_{g}",
                                     tag="fp")
                    nc.tensor.matmul(fps[:, 0:512], wt_conv[:],
                                     xsb[:, 0:512], start=True, stop=True)
                    nc.tensor.matmul(fps[:, 512:1024], wt_conv[:],
                                     xsb[:, 512:1024], start=True, stop=True)
                    glj = gl_p.tile([128, 1024], BF16, name=f"gl{b}_{g}",
                                    tag="gl")
                    nc.scalar.activation(
                        glj[:], fps[:], AF.Gelu, bias=bias_col[:],
                        accum_out=qacc[:, b * 4 + g:b * 4 + g + 1],
                    )
                if b == 7:
                    # q first: local normalize then AllGather immediately
                    qT = sml.tile([128, BL], F32)
                    nc.vector.tensor_reduce(
                        qT[:], qacc[:].rearrange("p (b g) -> p b g", g=4),
                        axis=mybir.AxisListType.X, op=ALU.add,
                    )
                    # mean over N so ||q||^2 lands in the rsqrt init range
                    nc.vector.tensor_scalar_mul(qT[:], qT[:], 1.0 / N)
                    qsq = sml.tile([128, BL], F32)
                    nc.vector.tensor_tensor(qsq[:], qT[:], qT[:],
                                            op=ALU.mult)
                    nrq = ft_ps.tile([1, BL], F32, name="nrq", tag="fp")
                    nc.tensor.matmul(nrq[:], ones[:, 0:1], qsq[:],
                                     start=True, stop=True)
                    nc.vector.tensor_copy(cwr[0:1, 0:BL], nrq[:])
                    # ||q||^2 concentrates near 3.0 (mean over 4096)
                    _rsqrt(nc, cwr[0:1, 16:16 + BL], cwr[0:1, 0:BL],
                           cwr[0:1, 32:32 + BL], 0.863, -0.0952,
                           iters=3)
                    cb = ft_ps.tile([128, BL], F32, name="cbq", tag="fp")
                    nc.tensor.matmul(cb[:], ones[0:1, :],
                                     cwr[0:1, 16:16 + BL],
                                     start=True, stop=True)
                    qcb = sml.tile([128, BL], F32)
                    nc.vector.tensor_tensor(qcb[:], qT[:], cb[:],
                                            op=ALU.mult)
                    nc.sync.dma_start(q_in[:], qcb[:])
                    # preload exp table while the AllGather flies
                    nc.scalar.activation(dume[:], qcb[0:1, 0:1], AF.Exp)
                # memory chunk work interleaved: 2 chunks per batch
                mchunk_sq(2 * b)
                mchunk_sq(2 * b + 1)
                if b % 2 == 1:
                    qt = b // 2               # quarter 0..3, chunks 4qt..4qt+3
                    sl = slice(qt * 16, qt * 16 + 16)
                    nc.vector.tensor_reduce(
                        nrm2[:, sl],
                        sqs[:, (qt % 2) * 16:(qt % 2) * 16 + 16],
                        axis=mybir.AxisListType.X, op=ALU.add,
                    )
                    _rsqrt(nc, minv[:, sl], nrm2[:, sl], mscr[:, sl],
                           0.1514, -3.715e-4)
                    for c in range(qt * 4, qt * 4 + 4):
                        mchunk_normalize(c)

        nc.gpsimd.collective_compute(
            "AllGather", ALU.bypass, replica_groups=groups,
            ins=[q_in[:]], outs=[q_ag[:]],
        )
        nc.sync.dma_start(
            qT_all[:].rearrange("p (c b) -> p c b", c=NCORES),
            q_ag[:].rearrange("(c p) b -> p c b", p=128),
        )
        nc.vector.tensor_copy(qTb[:], qT_all[:])

        # ---- sim matmuls (fold-2) + block max8, pipelined ----------------
        with tc.tile_pool(name="sim_ps", bufs=2, space="PSUM") as sim_psp:
            for cc in range(MC // 2):
                sp = sim_psp.tile([128, 512], F32)
                for half in range(2):
                    c = half * (MC // 2) + cc
                    nc.tensor.matmul(
                        sp[half * 64:half * 64 + 64, :],
                        qTb[:], mhT[:, c * 4:(c + 1) * 4].rearrange(
                            "p t d -> p (t d)"),
                        start=True, stop=True,
                    )
                if cc % 2 == 0:
                    nc.vector.tensor_copy(sim_sb[:, cc * 512:(cc + 1) * 512],
                                          sp[:])
                else:
                    nc.scalar.copy(sim_sb[:, cc * 512:(cc + 1) * 512], sp[:])
                    blk = cc // 2
                    nc.vector.max(
                        cands[:, blk * 8:(blk + 1) * 8],
                        sim_sb[:, blk * 1024:(blk + 1) * 1024],
                    )
        nc.sync.dma_start(cand_in[:, 0:32], cands[0:64, :])
        nc.sync.dma_start(cand_in[:, 32:64], cands[64:128, :])

        nc.gpsimd.collective_compute(
            "AllGather", ALU.bypass, replica_groups=groups,
            ins=[cand_in[:]], outs=[cand_ag[:]],
        )
        # dense exp during the AllGather flight (threshold-independent;
        # no scale needed -- q columns are already unit-norm)
        nc.scalar.activation(expw[:], sim_sb[:], AF.Exp)

        # read candidates into BOTH fold halves (skips a dup DMA later)
        for half in range(2):
            nc.sync.dma_start(
                cand_all[half * 64:half * 64 + 64].rearrange(
                    "b (c j) -> b c j", c=NCORES),
                cand_ag[:].rearrange("(c b) j -> b c j", b=B),
            )

        # ---- merge: global top-16 + threshold + Z ------------------------
        nc.vector.max(t16[:, 0:8], cand_all[:])
        nc.vector.match_replace(mr_scr[:], t16[:, 0:8], cand_all[:], -1.0e30)
        nc.vector.max(t16[:, 8:16], mr_scr[:])

        # Z_b = sum(exp(t16)); zscal = retrieval_scale / Z_b
        nc.scalar.activation(e16[:], t16[0:64, :], AF.Exp)
        nc.vector.tensor_reduce(cw[:, 0:1], e16[:], axis=mybir.AxisListType.X,
                                op=ALU.add)
        nc.vector.reciprocal(cw[:, 1:2], cw[:, 0:1])
        nc.vector.tensor_tensor(cw[:, 2:3], cw[:, 1:2], scal_col[0:64, 0:1],
                                op=ALU.mult)

        if debug:
            dbg_t16 = nc.dram_tensor("dbg_t16", [128, 16], F32,
                                     kind="ExternalOutput")
            dbg_qt = nc.dram_tensor("dbg_qt", [128, B], F32,
                                    kind="ExternalOutput")
            dbg_proto = nc.dram_tensor("dbg_proto", [B, D], F32,
                                       kind="ExternalOutput")
            dbg_sim = nc.dram_tensor("dbg_sim", [128, 4096], F32,
                                     kind="ExternalOutput")
            nc.sync.dma_start(dbg_t16[:], t16[:])
            nc.sync.dma_start(dbg_qt[:], qT_all[:])
            nc.sync.dma_start(dbg_proto[:], proto_sb[:])
            sim_f = big.tile([128, 4096], F32)
            nc.vector.tensor_copy(sim_f[:], sim_sb[:])
            nc.sync.dma_start(dbg_sim[:], sim_f[:])

        # ---- Phase D: masked W -> W^T (fold-2) -> proto ------------------
        with ExitStack() as pd:
            wt_sbp = pd.enter_context(tc.tile_pool(name="wt_sb", bufs=2))
            wt_psp = pd.enter_context(tc.tile_pool(name="wt_ps", bufs=2,
                                                   space="PSUM"))
            pr_ps = pd.enter_context(tc.tile_pool(name="pr_ps", bufs=1,
                                                  space="PSUM"))

            # bulk mask: two [128, 4096] bf16 DVE ops
            mk = big.tile([128, 4096], BF16)
            nc.vector.tensor_scalar(
                mk[:], sim_sb[:], t16[:, 15:16], None, op0=ALU.is_ge
            )
            nc.vector.tensor_tensor(expw[:], expw[:], mk[:], op=ALU.mult)

            # fold-2 transposes: expw[:, k-tile] -> [128s, 128] where
            # cols 0:64 = W^T for tile k, cols 64:128 = W^T for tile 32+k
            pr = pr_ps.tile([64, 128], F32)
            for k0 in range(0, 32, 8):
                wps = wt_psp.tile([128, 1024], BF16)
                for kk in range(8):
                    k = k0 + kk
                    nc.tensor.transpose(
                        wps[:, kk * 128:(kk + 1) * 128],
                        expw[:, k * 128:(k + 1) * 128],
                        identb[:],
                    )
                wsb = wt_sbp.tile([128, 1024], BF16)
                nc.vector.tensor_copy(wsb[:], wps[:])
                for kk in range(8):
                    k = k0 + kk
                    nc.tensor.matmul(
                        pr[:], wsb[:, kk * 128:kk * 128 + 64],
                        mraw[:, k], start=(k == 0), stop=False,
                    )
                    nc.tensor.matmul(
                        pr[:], wsb[:, kk * 128 + 64:(kk + 1) * 128],
                        mraw[:, 32 + k], start=False, stop=(k == 31),
                    )
            nc.vector.tensor_scalar_mul(proto_sb[:], pr[:], cw[:, 2:3])
            nc.sync.dma_start(proto_in[:], proto_sb[:])

        nc.gpsimd.collective_compute(
            "ReduceScatter", ALU.add, replica_groups=groups,
            ins=[proto_in[:]], outs=[proto_rs[:]],
        )
        nc.sync.dma_start(proto_loc[:], proto_rs[:].rearrange("b d -> (b d)")
                          .rearrange("(o f) -> o f", o=1))

        # ---- Phase E: out = x + proto broadcast --------------------------
        with ExitStack() as pe:
            bbp = pe.enter_context(tc.tile_pool(name="bb_ps", bufs=2,
                                                space="PSUM"))
            pbbp = pe.enter_context(tc.tile_pool(name="pbb", bufs=2))
            stgp = pe.enter_context(tc.tile_pool(name="stg", bufs=3))
            for b in range(BL):
                pb_ = bbp.tile([128, 128], F32)
                nc.tensor.matmul(pb_[:], ones[0:1, :],
                                 proto_loc[0:1, b * 128:(b + 1) * 128],
                                 start=True, stop=True)
                pbb = pbbp.tile([128, 128], BF16)
                nc.vector.tensor_copy(pbb[:], pb_[:])
                stg = stgp.tile([128, TPB, 128], BF16)
                nc.vector.tensor_tensor(
                    stg[:], xb[b][:],
                    pbb[:].rearrange("p (o d) -> p o d", o=1).broadcast_to(
                        [128, TPB, 128]),
                    op=ALU.add,
                )
                nc.gpsimd.dma_start(
                    out_ext[b].rearrange("(p t) d -> p t d", p=128), stg[:]
                )

    _hoist_waits(nc)
    return nc


_CACHED = {}


def kernel(x, conv_w, conv_b, memory, retrieval_scale):
    x = np.ascontiguousarray(np.asarray(x, dtype=np.float32))
    conv_w = np.ascontiguousarray(np.asarray(conv_w, dtype=np.float32))
    conv_b = np.ascontiguousarray(np.asarray(conv_b, dtype=np.float32))
    memory = np.ascontiguousarray(np.asarray(memory, dtype=np.float32))
    scal = np.asarray(retrieval_scale, dtype=np.float32).reshape(1)
    ident = np.eye(128, dtype=np.float32)
    import ml_dtypes
    identb = np.eye(128, dtype=ml_dtypes.bfloat16)

    if "nc" not in _CACHED:
        _CACHED["nc"] = build_program()
    nc = _CACHED["nc"]

    in_maps = []
    for c in range(NCORES):
        in_maps.append({
            "xs": x[c * BL:(c + 1) * BL],
            "ms": memory[c * SL:(c + 1) * SL],
            "convw": conv_w,
            "convb": conv_b,
            "scal": scal,
            "ident": ident,
            "identb": identb,
        })
    res = run_bass_kernel_spmd(nc, in_maps, list(range(NCORES)),
                               **_CACHED.get("run_kwargs", {}))
    _CACHED["last_result"] = res
    out = np.empty_like(x)
    for c in range(NCORES):
        out[c * BL:(c + 1) * BL] = res.results[c]["out"]
    return out
